# revision 24
# baseline (speedup 1.0000x reference)
"""Trainium2 Bass kernel for nn_CNNCacheModel (DilatedConvStack).

Model (reference.py): L=4 sandglass ConvBlocks over x[B=8, S=4096, D=1024]:
    res = x
    h = LayerNorm(x)                      (over D, eps=1e-5)
    h = causal depthwise conv(h)          (K=3, dilation 2**i, per-channel)
    h = gelu(h)
    h = gelu(h @ comp_w.T + comp_b)       (D -> DB=512)
    h = h @ exp_w.T + exp_b               (DB -> D)
    x = h + res

Sharding: data-parallel over batch B=8 across 8 NeuronCores (one sample per
core); conv/LN/matmuls are all per-sample so no collectives are needed.

Per-core layout: channels-on-partitions [D=part, S=free], host-pre-transposed.
LayerNorm stats (reductions across D = across partitions) are computed on the
TensorEngine with ones-vector matmuls (sum via float32r at 1 cyc/row, sum of
squares via bf16), then mean/rstd rows are broadcast back across partitions
with K=1 matmuls.  ln_scale/ln_bias are folded into the depthwise conv weights
on the host, so LN-apply is just (x - m) * r.  All GEMMs run in bf16 with fp32
PSUM accumulation; the residual x stays fp32 resident in SBUF for all 4
layers (only loaded/stored once).
"""

import sys

for p in ("/opt/trn_rl_repo",):
    if p not in sys.path:
        sys.path.insert(0, p)

import numpy as np
import ml_dtypes

import concourse.bass as bass
import concourse.bacc as bacc
import concourse.tile as tile
from concourse import mybir
from concourse.bass_utils import run_bass_kernel_spmd

F32 = mybir.dt.float32
F32R = mybir.dt.float32r
BF16 = mybir.dt.bfloat16
AF = mybir.ActivationFunctionType
OP = mybir.AluOpType

B, D, L, KTAPS, DB = 8, 1024, 4, 3, 512
EPS = 1e-5
NT = D // 128        # 8 D-tiles (partition groups)
NMC = DB // 128      # 4 compress output chunks
NTE = DB // 128      # 4 expand K-tiles
NME = D // 128       # 8 expand output chunks
HALO = 16            # (K-1) * max dilation = 2 * 8


def build_program(S=4096, Sc=512, stats_f32r=False, sim_safe=False):
    """Build the single-core Bass/Tile program (identical SPMD on all cores).

    sim_safe=True replaces the Gelu activation (not implemented in CoreSim)
    with x*sigmoid(1.702x); only used for simulator validation runs.
    """
    nc = bacc.Bacc("TRN2", target_bir_lowering=False, debug=False)
    NCH = S // Sc
    assert S % Sc == 0 and Sc >= 2 * HALO

    xt_d = nc.dram_tensor("xt", [D, S], F32, kind="ExternalInput")
    yt_d = nc.dram_tensor("yt", [D, S], F32, kind="ExternalOutput")
    dww_d = nc.dram_tensor("dww", [L, 128, NT, KTAPS], F32, kind="ExternalInput")
    dwb_d = nc.dram_tensor("dwb", [L, 128, NT], F32, kind="ExternalInput")
    cw_d = nc.dram_tensor("cw", [L, 128, NT, DB], BF16, kind="ExternalInput")
    cb_d = nc.dram_tensor("cb", [L, 128, NMC], F32, kind="ExternalInput")
    ew_d = nc.dram_tensor("ew", [L, 128, NTE, D], BF16, kind="ExternalInput")
    eb_d = nc.dram_tensor("eb", [L, 128, NME], F32, kind="ExternalInput")

    with tile.TileContext(nc) as tc:
        with (
            tc.tile_pool(name="xres", bufs=1) as xpool,
            tc.tile_pool(name="w", bufs=1) as wpool,
            tc.tile_pool(name="cons", bufs=1) as conspool,
            tc.tile_pool(name="rows", bufs=2) as rowp,
            tc.tile_pool(name="sv", bufs=1) as svp,
            tc.tile_pool(name="xq", bufs=2) as xqp,
            tc.tile_pool(name="xn", bufs=2) as xnp,
            tc.tile_pool(name="tmp", bufs=2) as tp,
            tc.tile_pool(name="h", bufs=2) as hp,
            tc.tile_pool(name="hc", bufs=2) as hcp,
            tc.tile_pool(name="bc", bufs=2) as bcp,
            tc.tile_pool(name="gelutmp", bufs=2) as gtp,
            tc.tile_pool(name="ps", bufs=8, space="PSUM") as psp,
        ):
            _gelu_n = [0]

            def emit_gelu(out, in_, bias_ap):
                if not sim_safe:
                    nc.scalar.activation(out, in_, AF.Gelu, bias=bias_ap)
                    return
                _gelu_n[0] += 1
                shp = list(in_.shape)
                tg1 = gtp.tile(shp, F32, tag="tg1", name=f"tg1_{_gelu_n[0]}")
                nc.scalar.activation(tg1, in_, AF.Identity, bias=bias_ap)
                tg2 = gtp.tile(shp, F32, tag="tg2", name=f"tg2_{_gelu_n[0]}")
                nc.scalar.activation(tg2, tg1, AF.Sigmoid, scale=1.702)
                nc.vector.tensor_mul(out, tg1, tg2)
            ones_bf = conspool.tile([128, 128], BF16)
            nc.vector.memset(ones_bf, 1.0)
            epsb = conspool.tile([128, 1], F32)
            nc.vector.memset(epsb, EPS)

            xres = []
            for t in range(NT):
                xt_ = xpool.tile([128, S], F32, tag=f"x{t}")
                nc.sync.dma_start(out=xt_, in_=xt_d.ap()[t * 128:(t + 1) * 128, :])
                xres.append(xt_)

            for li in range(L):
                dil = 2 ** li
                cw = wpool.tile([128, NT, DB], BF16, tag="cw")
                nc.sync.dma_start(out=cw, in_=cw_d.ap()[li])
                ew = wpool.tile([128, NTE, D], BF16, tag="ew")
                nc.sync.dma_start(out=ew, in_=ew_d.ap()[li])
                dww = wpool.tile([128, NT, KTAPS], F32, tag="dww")
                nc.sync.dma_start(out=dww, in_=dww_d.ap()[li])
                dwb = wpool.tile([128, NT], F32, tag="dwb")
                nc.sync.dma_start(out=dwb, in_=dwb_d.ap()[li])
                cb = wpool.tile([128, NMC], F32, tag="cb")
                nc.sync.dma_start(out=cb, in_=cb_d.ap()[li])
                eb = wpool.tile([128, NME], F32, tag="eb")
                nc.sync.dma_start(out=eb, in_=eb_d.ap()[li])

                # ---- Pass 1: LN statistics for every chunk (PE reductions) ----
                # Chunk c's sum / sum-of-squares rows live at partition
                # 32*(c%4) of PSUM bank c//4 (the only matmul-legal output
                # partitions).  The small-vector math then runs on whole
                # [128, Sc] banks: same instruction cost, and the unused
                # partitions are kept finite by the memsets below.
                nbank = (NCH + 3) // 4
                sb = []
                qb = []
                for bk in range(nbank):
                    sbt = psp.tile([128, Sc], F32, tag="ps", name=f"sb{li}_{bk}")
                    nc.vector.memset(sbt, 0.0)
                    sb.append(sbt)
                    qbt = psp.tile([128, Sc], F32, tag="ps", name=f"qb{li}_{bk}")
                    nc.vector.memset(qbt, float(D))
                    qb.append(qbt)
                for c in range(NCH):
                    lo = c * Sc
                    row = 32 * (c % 4)
                    ps_s = sb[c // 4][row:row + 1, :]
                    ps_q = qb[c // 4][row:row + 1, :]
                    for t in range(NT):
                        xsl = xres[t][:, lo:lo + Sc]
                        xq = xqp.tile([128, Sc], BF16, tag="xq")
                        nc.scalar.activation(xq, xsl, AF.Square)
                        xb = xqp.tile([128, Sc], BF16, tag="xb")
                        nc.gpsimd.tensor_copy(xb, xsl)
                        nc.tensor.matmul(
                            ps_s, ones_bf[:, 0:1], xb,
                            start=(t == 0), stop=(t == NT - 1),
                            tile_position=(0, row))
                        nc.tensor.matmul(
                            ps_q, ones_bf[:, 0:1], xq,
                            start=(t == 0), stop=(t == NT - 1),
                            tile_position=(0, row))

                # ---- batched small-vector math: mean/rstd for all chunks ----
                r_all = []
                mr_all = []
                for bk in range(nbank):
                    m_ = svp.tile([128, Sc], F32, tag="m", name=f"m{li}_{bk}")
                    nc.scalar.activation(m_, sb[bk], AF.Copy, scale=1.0 / D)
                    msq = svp.tile([128, Sc], F32, tag="msq", name=f"msq{li}_{bk}")
                    nc.scalar.activation(msq, m_, AF.Square)
                    # ve = sq/D - m^2, then lg = ln(ve+eps), in place in PSUM
                    nc.vector.scalar_tensor_tensor(
                        qb[bk], qb[bk], 1.0 / D, msq, op0=OP.mult, op1=OP.subtract)
                    nc.scalar.activation(qb[bk], qb[bk], AF.Ln, bias=epsb[:, 0:1])
                    ra = rowp.tile([128, Sc], BF16, tag="r_all", name=f"ra{li}_{bk}")
                    nc.scalar.activation(ra, qb[bk], AF.Exp, scale=-0.5)
                    r_all.append(ra)
                    mra = rowp.tile([128, Sc], BF16, tag="mr_all", name=f"mra{li}_{bk}")
                    nc.vector.tensor_mul(mra, m_, ra)
                    mr_all.append(mra)

                # ---- Pass 2: LN apply, conv, gelu, compress, expand, residual ----
                xn_prev = None
                for c in range(NCH):
                    lo = c * Sc
                    row = 32 * (c % 4)
                    r0 = r_all[c // 4][row:row + 1, :]
                    mr0 = mr_all[c // 4][row:row + 1, :]
                    rb_ps = psp.tile([128, Sc], F32, tag="ps")
                    nc.tensor.matmul(rb_ps, ones_bf[row:row + 1, :], r0,
                                     start=True, stop=True, tile_position=(row, 0))
                    mrb_ps = psp.tile([128, Sc], F32, tag="ps")
                    nc.tensor.matmul(mrb_ps, ones_bf[row:row + 1, :], mr0,
                                     start=True, stop=True, tile_position=(row, 0))
                    rbs = bcp.tile([128, Sc], BF16, tag="rbs")
                    nc.vector.tensor_copy(rbs, rb_ps)
                    mrbs = bcp.tile([128, Sc], BF16, tag="mrbs")
                    nc.vector.tensor_copy(mrbs, mrb_ps)

                    xn = xnp.tile([128, NT, HALO + Sc], BF16, tag="xn")
                    cps = [psp.tile([128, Sc], F32, tag="ps", name=f"cps{li}_{c}_{m}")
                           for m in range(NMC)]
                    for t in range(NT):
                        if c == 0:
                            nc.gpsimd.memset(xn[:, t, 0:HALO], 0.0)
                        else:
                            nc.vector.tensor_copy(
                                xn[:, t, 0:HALO], xn_prev[:, t, Sc:Sc + HALO])
                        tt_ = tp.tile([128, Sc], BF16, tag="tt")
                        nc.gpsimd.tensor_mul(tt_, xres[t][:, lo:lo + Sc], rbs)
                        nc.gpsimd.tensor_sub(xn[:, t, HALO:HALO + Sc], tt_, mrbs)
                        h = hp.tile([128, Sc], BF16, tag="h")
                        nc.vector.tensor_scalar(
                            h, xn[:, t, HALO - 2 * dil:HALO - 2 * dil + Sc],
                            dww[:, t, 0:1], None, op0=OP.mult)
                        nc.vector.scalar_tensor_tensor(
                            h, xn[:, t, HALO - dil:HALO - dil + Sc],
                            dww[:, t, 1:2], h, op0=OP.mult, op1=OP.add)
                        nc.vector.scalar_tensor_tensor(
                            h, xn[:, t, HALO:HALO + Sc],
                            dww[:, t, 2:3], h, op0=OP.mult, op1=OP.add)
                        emit_gelu(h, h, dwb[:, t:t + 1])
                        for m in range(NMC):
                            nc.tensor.matmul(
                                cps[m], cw[:, t, m * 128:(m + 1) * 128], h,
                                start=(t == 0), stop=(t == NT - 1))
                    xn_prev = xn

                    hc = hcp.tile([128, NTE, Sc], BF16, tag="hc")
                    for m in range(NMC):
                        emit_gelu(hc[:, m, :], cps[m], cb[:, m:m + 1])
                    for mo in range(NME):
                        ep = psp.tile([128, Sc], F32, tag="ps")
                        for e in range(NTE):
                            nc.tensor.matmul(
                                ep, ew[:, e, mo * 128:(mo + 1) * 128], hc[:, e, :],
                                start=(e == 0), stop=(e == NTE - 1))
                        nc.vector.scalar_tensor_tensor(
                            xres[mo][:, lo:lo + Sc], ep, eb[:, mo:mo + 1],
                            xres[mo][:, lo:lo + Sc], op0=OP.add, op1=OP.add)

            for t in range(NT):
                nc.sync.dma_start(
                    out=yt_d.ap()[t * 128:(t + 1) * 128, :], in_=xres[t])

    nc.compile()
    return nc


def host_prep(ln_scale, ln_bias, dw_w, dw_b, comp_w, comp_b, exp_w, exp_b):
    """Fold LN affine into conv weights and lay everything out device-friendly."""
    ln_scale = np.asarray(ln_scale, np.float32)
    ln_bias = np.asarray(ln_bias, np.float32)
    dw_w = np.asarray(dw_w, np.float32)
    dw_b = np.asarray(dw_b, np.float32)
    comp_w = np.asarray(comp_w, np.float32)
    comp_b = np.asarray(comp_b, np.float32)
    exp_w = np.asarray(exp_w, np.float32)
    exp_b = np.asarray(exp_b, np.float32)

    dww = dw_w * ln_scale[:, :, None]                       # [L, D, K]
    dwb = dw_b + ln_bias * dw_w.sum(-1)                     # [L, D]
    bf = ml_dtypes.bfloat16
    return {
        "dww": np.ascontiguousarray(
            dww.reshape(L, NT, 128, KTAPS).transpose(0, 2, 1, 3)),
        "dwb": np.ascontiguousarray(dwb.reshape(L, NT, 128).transpose(0, 2, 1)),
        "cw": np.ascontiguousarray(
            comp_w.transpose(0, 2, 1).reshape(L, NT, 128, DB)
            .transpose(0, 2, 1, 3)).astype(bf),
        "cb": np.ascontiguousarray(comp_b.reshape(L, NMC, 128).transpose(0, 2, 1)),
        "ew": np.ascontiguousarray(
            exp_w.transpose(0, 2, 1).reshape(L, NTE, 128, D)
            .transpose(0, 2, 1, 3)).astype(bf),
        "eb": np.ascontiguousarray(exp_b.reshape(L, NME, 128).transpose(0, 2, 1)),
    }


_CACHE = {}


def _get_program():
    if "nc" not in _CACHE:
        _CACHE["nc"] = build_program()
    return _CACHE["nc"]


def kernel(**inputs):
    x = np.asarray(inputs["x"], np.float32)                 # [B, S, D]
    w = host_prep(
        inputs["ln_scale"], inputs["ln_bias"], inputs["dw_w"], inputs["dw_b"],
        inputs["comp_w"], inputs["comp_b"], inputs["exp_w"], inputs["exp_b"])
    in_maps = []
    for core in range(B):
        m = dict(w)
        m["xt"] = np.ascontiguousarray(x[core].T)           # [D, S]
        in_maps.append(m)
    res = run_bass_kernel_spmd(_get_program(), in_maps, list(range(B)))
    return np.stack([res.results[i]["yt"].T for i in range(B)], axis=0)


# revision 29
# speedup vs baseline: 1.6393x; 1.6393x over previous
"""Trainium2 Bass kernel for nn_CNNCacheModel (DilatedConvStack).

Model (reference.py): L=4 sandglass ConvBlocks over x[B=8, S=4096, D=1024]:
    res = x
    h = LayerNorm(x)                      (over D, eps=1e-5)
    h = causal depthwise conv(h)          (K=3, dilation 2**i, per-channel)
    h = gelu(h)
    h = gelu(h @ comp_w.T + comp_b)       (D -> DB=512)
    h = h @ exp_w.T + exp_b               (DB -> D)
    x = h + res

Sharding: data-parallel over batch B=8 across 8 NeuronCores (one sample per
core); conv/LN/matmuls are all per-sample so no collectives are needed.

Per-core layout: channels-on-partitions [D=part, S=free], host-pre-transposed.
Engine assignment (calibrated from a perfetto trace of v1):
  - PE: all GEMMs (bf16, fp32 PSUM), the depthwise conv as 3 diagonal-weight
    matmuls per D-tile, LayerNorm sum-of-squares reductions via ones-vector
    matmuls, per-chunk mean/rstd broadcast via K=1 matmuls, and incremental
    mean updates via column-sum matmuls over the expand activations.
  - DVE: x->bf16 casts, LN apply (2 bf16 tensor_tensor ops), residual add.
  - ACT: gelu (fused per-channel bias), PSUM->SBUF broadcast copies,
    rstd math (ln/exp, batched per layer to avoid ACT table-set thrash).
  - GPSIMD: x^2 squares and tiny halo copies only (it is slow per op).
LayerNorm statistics live at matmul-legal partitions {0,32,64,96} of shared
PSUM banks; the mean is tracked incrementally across layers:
    sum_d x_new = sum_d x_old + colsum(exp_w) @ hc + sum(exp_b).
ln_scale/ln_bias are folded into the conv weights on the host.
"""

import sys

for p in ("/opt/trn_rl_repo",):
    if p not in sys.path:
        sys.path.insert(0, p)

import numpy as np
import ml_dtypes

import concourse.bass as bass
import concourse.bacc as bacc
import concourse.tile as tile
from concourse import mybir
from concourse.bass_utils import run_bass_kernel_spmd

F32 = mybir.dt.float32
BF16 = mybir.dt.bfloat16
AF = mybir.ActivationFunctionType
OP = mybir.AluOpType

B, D, L, KTAPS, DB = 8, 1024, 4, 3, 512
EPS = 1e-5
NT = D // 128        # 8 D-tiles (partition groups)
NMC = DB // 128      # 4 compress output chunks
NTE = DB // 128      # 4 expand K-tiles
NME = D // 128       # 8 expand output chunks
HALO = 16            # (K-1) * max dilation = 2 * 8


def build_program(S=4096, Sc=512, sim_safe=False):
    """Build the single-core Bass/Tile program (identical SPMD on all cores).

    sim_safe=True replaces the Gelu activation (not implemented in CoreSim)
    with x*sigmoid(1.702x); only used for simulator validation runs.
    """
    nc = bacc.Bacc("TRN2", target_bir_lowering=False, debug=False)
    NCH = S // Sc
    assert S % Sc == 0 and Sc >= 2 * HALO
    nbank = (NCH + 3) // 4

    xt_d = nc.dram_tensor("xt", [D, S], F32, kind="ExternalInput")
    yt_d = nc.dram_tensor("yt", [D, S], F32, kind="ExternalOutput")
    dwd_d = nc.dram_tensor("dwd", [L, 128, NT, KTAPS, 128], BF16,
                           kind="ExternalInput")
    dwb_d = nc.dram_tensor("dwb", [L, 128, NT], F32, kind="ExternalInput")
    cw_d = nc.dram_tensor("cw", [L, 128, NT, DB], BF16, kind="ExternalInput")
    cb_d = nc.dram_tensor("cb", [L, 128, NMC], F32, kind="ExternalInput")
    ew_d = nc.dram_tensor("ew", [L, 128, NTE, D], BF16, kind="ExternalInput")
    eb_d = nc.dram_tensor("eb", [L, 128, NME], F32, kind="ExternalInput")
    ecs_d = nc.dram_tensor("ecs", [L, 128, NTE], BF16, kind="ExternalInput")
    ebs_d = nc.dram_tensor("ebs", [L, 128, 1], F32, kind="ExternalInput")

    with tile.TileContext(nc) as tc:
        with (
            tc.tile_pool(name="xres", bufs=1) as xpool,
            tc.tile_pool(name="w", bufs=1) as wpool,
            tc.tile_pool(name="cons", bufs=1) as conspool,
            tc.tile_pool(name="rows", bufs=2) as rowp,
            tc.tile_pool(name="sv", bufs=1) as svp,
            tc.tile_pool(name="xq", bufs=3) as xqp,
            tc.tile_pool(name="xn", bufs=2) as xnp,
            tc.tile_pool(name="tmp", bufs=3) as tp,
            tc.tile_pool(name="h", bufs=3) as hp,
            tc.tile_pool(name="hc", bufs=2) as hcp,
            tc.tile_pool(name="bc", bufs=2) as bcp,
            tc.tile_pool(name="gelutmp", bufs=2) as gtp,
            tc.tile_pool(name="ps", bufs=8, space="PSUM") as psp,
        ):
            _gelu_n = [0]

            def emit_gelu(out, in_, bias_ap):
                if not sim_safe:
                    nc.scalar.activation(out, in_, AF.Gelu, bias=bias_ap)
                    return
                _gelu_n[0] += 1
                shp = list(in_.shape)
                tg1 = gtp.tile(shp, F32, tag="tg1", name=f"tg1_{_gelu_n[0]}")
                nc.scalar.activation(tg1, in_, AF.Identity, bias=bias_ap)
                tg2 = gtp.tile(shp, F32, tag="tg2", name=f"tg2_{_gelu_n[0]}")
                nc.scalar.activation(tg2, tg1, AF.Sigmoid, scale=1.702)
                nc.vector.tensor_mul(out, tg1, tg2)

            ones_bf = conspool.tile([128, 128], BF16)
            nc.gpsimd.memset(ones_bf, 1.0)
            epsb = conspool.tile([128, 1], F32)
            nc.gpsimd.memset(epsb, EPS)
            # running mean, one [128, Sc] tile per stats bank (rows at
            # partitions {0,32,64,96} hold chunks 4*bk .. 4*bk+3)
            ms = []
            for bk in range(nbank):
                mst = conspool.tile([128, Sc], F32, name=f"ms{bk}")
                ms.append(mst)

            xres = []
            for t in range(NT):
                xt_ = xpool.tile([128, S], F32, tag=f"x{t}")
                nc.sync.dma_start(out=xt_, in_=xt_d.ap()[t * 128:(t + 1) * 128, :])
                xres.append(xt_)

            delta_banks = None
            for li in range(L):
                dil = 2 ** li
                dwd = wpool.tile([128, NT, KTAPS, 128], BF16, tag="dwd")
                nc.sync.dma_start(out=dwd, in_=dwd_d.ap()[li])
                cw = wpool.tile([128, NT, DB], BF16, tag="cw")
                nc.sync.dma_start(out=cw, in_=cw_d.ap()[li])
                ew = wpool.tile([128, NTE, D], BF16, tag="ew")
                nc.sync.dma_start(out=ew, in_=ew_d.ap()[li])
                dwb = wpool.tile([128, NT], F32, tag="dwb")
                nc.sync.dma_start(out=dwb, in_=dwb_d.ap()[li])
                cb = wpool.tile([128, NMC], F32, tag="cb")
                nc.sync.dma_start(out=cb, in_=cb_d.ap()[li])
                eb = wpool.tile([128, NME], F32, tag="eb")
                nc.sync.dma_start(out=eb, in_=eb_d.ap()[li])
                ecs = wpool.tile([128, NTE], BF16, tag="ecs")
                nc.sync.dma_start(out=ecs, in_=ecs_d.ap()[li])
                ebs = wpool.tile([128, 1], F32, tag="ebs")
                nc.sync.dma_start(out=ebs, in_=ebs_d.ap()[li])

                # ---- Pass 1: sum-of-squares for every chunk (PE reductions);
                # layer 0 additionally reduces the plain sum for the mean. ----
                qb = []
                sb = []
                for bk in range(nbank):
                    qbt = psp.tile([128, Sc], F32, tag="ps", name=f"qb{li}_{bk}")
                    nc.vector.memset(qbt, float(D))
                    qb.append(qbt)
                    if li == 0:
                        sbt = psp.tile([128, Sc], F32, tag="ps", name=f"sb{li}_{bk}")
                        nc.vector.memset(sbt, 0.0)
                        sb.append(sbt)
                for c in range(NCH):
                    lo = c * Sc
                    row = 32 * (c % 4)
                    bk = c // 4
                    for t in range(NT):
                        xsl = xres[t][:, lo:lo + Sc]
                        xq = xqp.tile([128, Sc], BF16, tag="xq")
                        nc.gpsimd.tensor_mul(xq, xsl, xsl)
                        nc.tensor.matmul(
                            qb[bk][row:row + 1, :], ones_bf[:, 0:1], xq,
                            start=(t == 0), stop=(t == NT - 1),
                            tile_position=(0, row))
                        if li == 0:
                            xb = xqp.tile([128, Sc], BF16, tag="xb")
                            nc.vector.tensor_copy(xb, xsl)
                            nc.tensor.matmul(
                                sb[bk][row:row + 1, :], ones_bf[:, 0:1], xb,
                                start=(t == 0), stop=(t == NT - 1),
                                tile_position=(0, row))

                # ---- mean/rstd math, batched on whole stats banks ----
                r_all = []
                mr_all = []
                for bk in range(nbank):
                    if li == 0:
                        nc.scalar.activation(ms[bk], sb[bk], AF.Copy, scale=1.0 / D)
                    else:
                        # mean += (colsum(exp_w) @ hc + sum(exp_b)) / D
                        nc.vector.scalar_tensor_tensor(
                            ms[bk], delta_banks[bk], 1.0 / D, ms[bk],
                            op0=OP.mult, op1=OP.add)
                        nc.scalar.activation(ms[bk], ms[bk], AF.Identity,
                                             bias=ebs[:, 0:1])
                    msq = svp.tile([128, Sc], F32, tag="msq", name=f"msq{li}_{bk}")
                    nc.scalar.activation(msq, ms[bk], AF.Square)
                    # var = sq/D - m^2 ; rstd = exp(-0.5*ln(var+eps)), in PSUM
                    nc.vector.scalar_tensor_tensor(
                        qb[bk], qb[bk], 1.0 / D, msq, op0=OP.mult, op1=OP.subtract)
                    nc.scalar.activation(qb[bk], qb[bk], AF.Ln, bias=epsb[:, 0:1])
                    ra = rowp.tile([128, Sc], BF16, tag="r_all", name=f"ra{li}_{bk}")
                    nc.scalar.activation(ra, qb[bk], AF.Exp, scale=-0.5)
                    r_all.append(ra)
                    mra = rowp.tile([128, Sc], BF16, tag="mr_all", name=f"mra{li}_{bk}")
                    nc.vector.tensor_mul(mra, ms[bk], ra)
                    mr_all.append(mra)

                # delta banks for the NEXT layer's mean update
                new_delta = None
                if li < L - 1:
                    new_delta = []
                    for bk in range(nbank):
                        dbt = psp.tile([128, Sc], F32, tag="ps", name=f"db{li}_{bk}")
                        nc.vector.memset(dbt, 0.0)
                        new_delta.append(dbt)

                # ---- Pass 2: LN apply, conv, gelu, compress, expand, residual ----
                xn_prev = None
                for c in range(NCH):
                    lo = c * Sc
                    row = 32 * (c % 4)
                    bk = c // 4
                    r0 = r_all[bk][row:row + 1, :]
                    mr0 = mr_all[bk][row:row + 1, :]
                    rb_ps = psp.tile([128, Sc], F32, tag="ps")
                    nc.tensor.matmul(rb_ps, ones_bf[row:row + 1, :], r0,
                                     start=True, stop=True, tile_position=(row, 0))
                    mrb_ps = psp.tile([128, Sc], F32, tag="ps")
                    nc.tensor.matmul(mrb_ps, ones_bf[row:row + 1, :], mr0,
                                     start=True, stop=True, tile_position=(row, 0))
                    rbs = bcp.tile([128, Sc], BF16, tag="rbs")
                    nc.scalar.copy(rbs, rb_ps)
                    mrbs = bcp.tile([128, Sc], BF16, tag="mrbs")
                    nc.scalar.copy(mrbs, mrb_ps)

                    xn = xnp.tile([128, NT, HALO + Sc], BF16, tag="xn")
                    cps = [psp.tile([128, Sc], F32, tag="ps", name=f"cps{li}_{c}_{m}")
                           for m in range(NMC)]
                    for t in range(NT):
                        if c == 0:
                            nc.gpsimd.memset(xn[:, t, 0:HALO], 0.0)
                        else:
                            nc.gpsimd.tensor_copy(
                                xn[:, t, 0:HALO], xn_prev[:, t, Sc:Sc + HALO])
                        xb2 = tp.tile([128, Sc], BF16, tag="xb2")
                        nc.vector.tensor_copy(xb2, xres[t][:, lo:lo + Sc])
                        tt_ = tp.tile([128, Sc], BF16, tag="tt")
                        nc.vector.tensor_mul(tt_, xb2, rbs)
                        nc.vector.tensor_sub(
                            xn[:, t, HALO:HALO + Sc], tt_, mrbs)
                        # depthwise conv: 3 diagonal-weight matmuls into PSUM
                        cv = psp.tile([128, Sc], F32, tag="ps",
                                      name=f"cv{li}_{c}_{t}")
                        for k in range(KTAPS):
                            off = HALO - (KTAPS - 1 - k) * dil
                            nc.tensor.matmul(
                                cv, dwd[:, t, k, :], xn[:, t, off:off + Sc],
                                start=(k == 0), stop=(k == KTAPS - 1))
                        h = hp.tile([128, Sc], BF16, tag="h")
                        emit_gelu(h, cv, dwb[:, t:t + 1])
                        for m in range(NMC):
                            nc.tensor.matmul(
                                cps[m], cw[:, t, m * 128:(m + 1) * 128], h,
                                start=(t == 0), stop=(t == NT - 1))
                    xn_prev = xn

                    hc = hcp.tile([128, NTE, Sc], BF16, tag="hc")
                    for m in range(NMC):
                        emit_gelu(hc[:, m, :], cps[m], cb[:, m:m + 1])
                    if new_delta is not None:
                        for e in range(NTE):
                            nc.tensor.matmul(
                                new_delta[bk][row:row + 1, :], ecs[:, e:e + 1],
                                hc[:, e, :], start=(e == 0), stop=(e == NTE - 1),
                                tile_position=(0, row))
                    for mo in range(NME):
                        ep = psp.tile([128, Sc], F32, tag="ps")
                        for e in range(NTE):
                            nc.tensor.matmul(
                                ep, ew[:, e, mo * 128:(mo + 1) * 128], hc[:, e, :],
                                start=(e == 0), stop=(e == NTE - 1))
                        nc.vector.scalar_tensor_tensor(
                            xres[mo][:, lo:lo + Sc], ep, eb[:, mo:mo + 1],
                            xres[mo][:, lo:lo + Sc], op0=OP.add, op1=OP.add)
                delta_banks = new_delta

            for t in range(NT):
                nc.sync.dma_start(
                    out=yt_d.ap()[t * 128:(t + 1) * 128, :], in_=xres[t])

    nc.compile()
    return nc


def host_prep(ln_scale, ln_bias, dw_w, dw_b, comp_w, comp_b, exp_w, exp_b):
    """Fold LN affine into conv weights and lay everything out device-friendly."""
    ln_scale = np.asarray(ln_scale, np.float32)
    ln_bias = np.asarray(ln_bias, np.float32)
    dw_w = np.asarray(dw_w, np.float32)
    dw_b = np.asarray(dw_b, np.float32)
    comp_w = np.asarray(comp_w, np.float32)
    comp_b = np.asarray(comp_b, np.float32)
    exp_w = np.asarray(exp_w, np.float32)
    exp_b = np.asarray(exp_b, np.float32)

    dww = dw_w * ln_scale[:, :, None]                       # [L, D, K]
    dwb = dw_b + ln_bias * dw_w.sum(-1)                     # [L, D]
    bf = ml_dtypes.bfloat16
    # diagonal conv weights: dwd[l, p, t, k, p] = dww[l, t*128+p, k]
    dww_ptk = dww.reshape(L, NT, 128, KTAPS).transpose(0, 2, 1, 3)  # [L,128,NT,K]
    dwd = np.zeros((L, 128, NT, KTAPS, 128), np.float32)
    idx = np.arange(128)
    dwd[:, idx, :, :, idx] = dww_ptk.transpose(1, 0, 2, 3)
    ecs = exp_w.sum(1)                                      # [L, DB]
    # ebs[l] is consumed at layer l for the delta produced by layer l-1's
    # expand, so shift the per-layer bias sums by one layer.
    ebs = np.concatenate([[0.0], exp_b.sum(-1)[:-1] / D]).astype(np.float32)
    return {
        "dwd": np.ascontiguousarray(dwd).astype(bf),
        "dwb": np.ascontiguousarray(dwb.reshape(L, NT, 128).transpose(0, 2, 1)),
        "cw": np.ascontiguousarray(
            comp_w.transpose(0, 2, 1).reshape(L, NT, 128, DB)
            .transpose(0, 2, 1, 3)).astype(bf),
        "cb": np.ascontiguousarray(comp_b.reshape(L, NMC, 128).transpose(0, 2, 1)),
        "ew": np.ascontiguousarray(
            exp_w.transpose(0, 2, 1).reshape(L, NTE, 128, D)
            .transpose(0, 2, 1, 3)).astype(bf),
        "eb": np.ascontiguousarray(exp_b.reshape(L, NME, 128).transpose(0, 2, 1)),
        "ecs": np.ascontiguousarray(ecs.reshape(L, NTE, 128).transpose(0, 2, 1))
        .astype(bf),
        "ebs": np.broadcast_to(ebs[:, None, None], (L, 128, 1)).copy(),
    }


_CACHE = {}


def _get_program():
    if "nc" not in _CACHE:
        _CACHE["nc"] = build_program()
    return _CACHE["nc"]


def kernel(**inputs):
    x = np.asarray(inputs["x"], np.float32)                 # [B, S, D]
    w = host_prep(
        inputs["ln_scale"], inputs["ln_bias"], inputs["dw_w"], inputs["dw_b"],
        inputs["comp_w"], inputs["comp_b"], inputs["exp_w"], inputs["exp_b"])
    in_maps = []
    for core in range(B):
        m = dict(w)
        m["xt"] = np.ascontiguousarray(x[core].T)           # [D, S]
        in_maps.append(m)
    res = run_bass_kernel_spmd(_get_program(), in_maps, list(range(B)))
    return np.stack([res.results[i]["yt"].T for i in range(B)], axis=0)


# revision 32
# speedup vs baseline: 1.7013x; 1.0378x over previous
"""Trainium2 Bass kernel for nn_CNNCacheModel (DilatedConvStack).

Model (reference.py): L=4 sandglass ConvBlocks over x[B=8, S=4096, D=1024]:
    res = x
    h = LayerNorm(x)                      (over D, eps=1e-5)
    h = causal depthwise conv(h)          (K=3, dilation 2**i, per-channel)
    h = gelu(h)
    h = gelu(h @ comp_w.T + comp_b)       (D -> DB=512)
    h = h @ exp_w.T + exp_b               (DB -> D)
    x = h + res

Sharding: data-parallel over batch B=8 across 8 NeuronCores (one sample per
core); conv/LN/matmuls are all per-sample so no collectives are needed.

Per-core layout: channels-on-partitions [D=part, S=free], host-pre-transposed.
Engine assignment (calibrated from a perfetto trace of v1):
  - PE: all GEMMs (bf16, fp32 PSUM), the depthwise conv as 3 diagonal-weight
    matmuls per D-tile, LayerNorm sum-of-squares reductions via ones-vector
    matmuls, per-chunk mean/rstd broadcast via K=1 matmuls, and incremental
    mean updates via column-sum matmuls over the expand activations.
  - DVE: x->bf16 casts, LN apply (2 bf16 tensor_tensor ops), residual add.
  - ACT: gelu (fused per-channel bias), PSUM->SBUF broadcast copies,
    rstd math (ln/exp, batched per layer to avoid ACT table-set thrash).
  - GPSIMD: x^2 squares and tiny halo copies only (it is slow per op).
LayerNorm statistics live at matmul-legal partitions {0,32,64,96} of shared
PSUM banks; the mean is tracked incrementally across layers:
    sum_d x_new = sum_d x_old + colsum(exp_w) @ hc + sum(exp_b).
ln_scale/ln_bias are folded into the conv weights on the host.
"""

import sys

for p in ("/opt/trn_rl_repo",):
    if p not in sys.path:
        sys.path.insert(0, p)

import numpy as np
import ml_dtypes

import concourse.bass as bass
import concourse.bacc as bacc
import concourse.tile as tile
from concourse import mybir
from concourse.bass_utils import run_bass_kernel_spmd

F32 = mybir.dt.float32
BF16 = mybir.dt.bfloat16
AF = mybir.ActivationFunctionType
OP = mybir.AluOpType

B, D, L, KTAPS, DB = 8, 1024, 4, 3, 512
EPS = 1e-5
NT = D // 128        # 8 D-tiles (partition groups)
NMC = DB // 128      # 4 compress output chunks
NTE = DB // 128      # 4 expand K-tiles
NME = D // 128       # 8 expand output chunks
HALO = 16            # (K-1) * max dilation = 2 * 8


def build_program(S=4096, Sc=512, sim_safe=False):
    """Build the single-core Bass/Tile program (identical SPMD on all cores).

    sim_safe=True replaces the Gelu activation (not implemented in CoreSim)
    with x*sigmoid(1.702x); only used for simulator validation runs.
    """
    nc = bacc.Bacc("TRN2", target_bir_lowering=False, debug=False)
    NCH = S // Sc
    assert S % Sc == 0 and Sc >= 2 * HALO
    nbank = (NCH + 3) // 4

    xt_d = nc.dram_tensor("xt", [D, S], F32, kind="ExternalInput")
    yt_d = nc.dram_tensor("yt", [D, S], F32, kind="ExternalOutput")
    dwd_d = nc.dram_tensor("dwd", [L, 128, NT, KTAPS, 128], BF16,
                           kind="ExternalInput")
    dwb_d = nc.dram_tensor("dwb", [L, 128, NT], F32, kind="ExternalInput")
    cw_d = nc.dram_tensor("cw", [L, 128, NT, DB], BF16, kind="ExternalInput")
    cb_d = nc.dram_tensor("cb", [L, 128, NMC], F32, kind="ExternalInput")
    ew_d = nc.dram_tensor("ew", [L, 128, NTE, D], BF16, kind="ExternalInput")
    eb_d = nc.dram_tensor("eb", [L, 128, NME], F32, kind="ExternalInput")
    ecs_d = nc.dram_tensor("ecs", [L, 128, NTE], BF16, kind="ExternalInput")
    ebs_d = nc.dram_tensor("ebs", [L, 128, 1], F32, kind="ExternalInput")

    with tile.TileContext(nc) as tc:
        with (
            tc.tile_pool(name="xres", bufs=1) as xpool,
            tc.tile_pool(name="w", bufs=1) as wpool,
            tc.tile_pool(name="cons", bufs=1) as conspool,
            tc.tile_pool(name="rows", bufs=2) as rowp,
            tc.tile_pool(name="sv", bufs=1) as svp,
            tc.tile_pool(name="xq", bufs=3) as xqp,
            tc.tile_pool(name="xn", bufs=2) as xnp,
            tc.tile_pool(name="tmp", bufs=3) as tp,
            tc.tile_pool(name="h", bufs=3) as hp,
            tc.tile_pool(name="hc", bufs=2) as hcp,
            tc.tile_pool(name="bc", bufs=2) as bcp,
            tc.tile_pool(name="gelutmp", bufs=2) as gtp,
            tc.tile_pool(name="ps", bufs=8, space="PSUM") as psp,
        ):
            _gelu_n = [0]

            def emit_gelu(out, in_, bias_ap):
                if not sim_safe:
                    nc.scalar.activation(out, in_, AF.Gelu, bias=bias_ap)
                    return
                _gelu_n[0] += 1
                shp = list(in_.shape)
                tg1 = gtp.tile(shp, F32, tag="tg1", name=f"tg1_{_gelu_n[0]}")
                nc.scalar.activation(tg1, in_, AF.Identity, bias=bias_ap)
                tg2 = gtp.tile(shp, F32, tag="tg2", name=f"tg2_{_gelu_n[0]}")
                nc.scalar.activation(tg2, tg1, AF.Sigmoid, scale=1.702)
                nc.vector.tensor_mul(out, tg1, tg2)

            ones_bf = conspool.tile([128, 128], BF16)
            nc.gpsimd.memset(ones_bf, 1.0)
            epsb = conspool.tile([128, 1], F32)
            nc.gpsimd.memset(epsb, EPS)
            # running mean, one [128, Sc] tile per stats bank (rows at
            # partitions {0,32,64,96} hold chunks 4*bk .. 4*bk+3)
            ms = []
            for bk in range(nbank):
                mst = conspool.tile([128, Sc], F32, name=f"ms{bk}")
                ms.append(mst)

            xres = []
            for t in range(NT):
                xt_ = xpool.tile([128, S], F32, tag=f"x{t}")
                for c in range(NCH):
                    lo = c * Sc
                    nc.sync.dma_start(
                        out=xt_[:, lo:lo + Sc],
                        in_=xt_d.ap()[t * 128:(t + 1) * 128, lo:lo + Sc])
                xres.append(xt_)

            delta_banks = None
            for li in range(L):
                dil = 2 ** li
                dwd = wpool.tile([128, NT, KTAPS, 128], BF16, tag="dwd")
                nc.sync.dma_start(out=dwd, in_=dwd_d.ap()[li])
                cw = wpool.tile([128, NT, DB], BF16, tag="cw")
                nc.sync.dma_start(out=cw, in_=cw_d.ap()[li])
                ew = wpool.tile([128, NTE, D], BF16, tag="ew")
                nc.sync.dma_start(out=ew, in_=ew_d.ap()[li])
                dwb = wpool.tile([128, NT], F32, tag="dwb")
                nc.sync.dma_start(out=dwb, in_=dwb_d.ap()[li])
                cb = wpool.tile([128, NMC], F32, tag="cb")
                nc.sync.dma_start(out=cb, in_=cb_d.ap()[li])
                eb = wpool.tile([128, NME], F32, tag="eb")
                nc.sync.dma_start(out=eb, in_=eb_d.ap()[li])
                ecs = wpool.tile([128, NTE], BF16, tag="ecs")
                nc.sync.dma_start(out=ecs, in_=ecs_d.ap()[li])
                ebs = wpool.tile([128, 1], F32, tag="ebs")
                nc.sync.dma_start(out=ebs, in_=ebs_d.ap()[li])

                # ---- Pass 1: sum-of-squares for every chunk (PE reductions);
                # layer 0 additionally reduces the plain sum for the mean. ----
                qb = []
                sb = []
                for bk in range(nbank):
                    qbt = psp.tile([128, Sc], F32, tag="ps", name=f"qb{li}_{bk}")
                    nc.vector.memset(qbt, float(D))
                    qb.append(qbt)
                    if li == 0:
                        sbt = psp.tile([128, Sc], F32, tag="ps", name=f"sb{li}_{bk}")
                        nc.vector.memset(sbt, 0.0)
                        sb.append(sbt)
                for c in range(NCH):
                    lo = c * Sc
                    row = 32 * (c % 4)
                    bk = c // 4
                    for t in range(NT):
                        xsl = xres[t][:, lo:lo + Sc]
                        xq = xqp.tile([128, Sc], BF16, tag="xq")
                        if t % 2 == 0:
                            nc.vector.tensor_mul(xq, xsl, xsl)
                        else:
                            nc.gpsimd.tensor_mul(xq, xsl, xsl)
                        nc.tensor.matmul(
                            qb[bk][row:row + 1, :], ones_bf[:, 0:1], xq,
                            start=(t == 0), stop=(t == NT - 1),
                            tile_position=(0, row))
                        if li == 0:
                            xb = xqp.tile([128, Sc], BF16, tag="xb")
                            nc.vector.tensor_copy(xb, xsl)
                            nc.tensor.matmul(
                                sb[bk][row:row + 1, :], ones_bf[:, 0:1], xb,
                                start=(t == 0), stop=(t == NT - 1),
                                tile_position=(0, row))

                # ---- mean/rstd math, batched on whole stats banks ----
                r_all = []
                mr_all = []
                for bk in range(nbank):
                    if li == 0:
                        nc.scalar.activation(ms[bk], sb[bk], AF.Copy, scale=1.0 / D)
                    else:
                        # mean += (colsum(exp_w) @ hc + sum(exp_b)) / D
                        nc.vector.scalar_tensor_tensor(
                            ms[bk], delta_banks[bk], 1.0 / D, ms[bk],
                            op0=OP.mult, op1=OP.add)
                        nc.scalar.activation(ms[bk], ms[bk], AF.Identity,
                                             bias=ebs[:, 0:1])
                    msq = svp.tile([128, Sc], F32, tag="msq", name=f"msq{li}_{bk}")
                    nc.vector.tensor_mul(msq, ms[bk], ms[bk])
                    # var = sq/D - m^2 ; rstd = exp(-0.5*ln(var+eps)), in PSUM
                    nc.vector.scalar_tensor_tensor(
                        qb[bk], qb[bk], 1.0 / D, msq, op0=OP.mult, op1=OP.subtract)
                    nc.scalar.activation(qb[bk], qb[bk], AF.Ln, bias=epsb[:, 0:1])
                    ra = rowp.tile([128, Sc], BF16, tag="r_all", name=f"ra{li}_{bk}")
                    nc.scalar.activation(ra, qb[bk], AF.Exp, scale=-0.5)
                    r_all.append(ra)
                    mra = rowp.tile([128, Sc], BF16, tag="mr_all", name=f"mra{li}_{bk}")
                    nc.vector.tensor_mul(mra, ms[bk], ra)
                    mr_all.append(mra)

                # delta banks for the NEXT layer's mean update
                new_delta = None
                if li < L - 1:
                    new_delta = []
                    for bk in range(nbank):
                        dbt = psp.tile([128, Sc], F32, tag="ps", name=f"db{li}_{bk}")
                        nc.vector.memset(dbt, 0.0)
                        new_delta.append(dbt)

                # ---- Pass 2: LN apply, conv, gelu, compress, expand, residual ----
                xn_prev = None
                for c in range(NCH):
                    lo = c * Sc
                    row = 32 * (c % 4)
                    bk = c // 4
                    r0 = r_all[bk][row:row + 1, :]
                    mr0 = mr_all[bk][row:row + 1, :]
                    rb_ps = psp.tile([128, Sc], F32, tag="ps")
                    nc.tensor.matmul(rb_ps, ones_bf[row:row + 1, :], r0,
                                     start=True, stop=True, tile_position=(row, 0))
                    mrb_ps = psp.tile([128, Sc], F32, tag="ps")
                    nc.tensor.matmul(mrb_ps, ones_bf[row:row + 1, :], mr0,
                                     start=True, stop=True, tile_position=(row, 0))
                    rbs = bcp.tile([128, Sc], BF16, tag="rbs")
                    nc.scalar.copy(rbs, rb_ps)
                    mrbs = bcp.tile([128, Sc], BF16, tag="mrbs")
                    nc.scalar.copy(mrbs, mrb_ps)

                    xn = xnp.tile([128, NT, HALO + Sc], BF16, tag="xn")
                    cps = [psp.tile([128, Sc], F32, tag="ps", name=f"cps{li}_{c}_{m}")
                           for m in range(NMC)]
                    for t in range(NT):
                        if c == 0:
                            nc.gpsimd.memset(xn[:, t, 0:HALO], 0.0)
                        else:
                            nc.gpsimd.tensor_copy(
                                xn[:, t, 0:HALO], xn_prev[:, t, Sc:Sc + HALO])
                        xb2 = tp.tile([128, Sc], BF16, tag="xb2")
                        nc.vector.tensor_copy(xb2, xres[t][:, lo:lo + Sc])
                        tt_ = tp.tile([128, Sc], BF16, tag="tt")
                        nc.vector.tensor_mul(tt_, xb2, rbs)
                        nc.vector.tensor_sub(
                            xn[:, t, HALO:HALO + Sc], tt_, mrbs)
                        # depthwise conv: 3 diagonal-weight matmuls into PSUM
                        cv = psp.tile([128, Sc], F32, tag="ps",
                                      name=f"cv{li}_{c}_{t}")
                        for k in range(KTAPS):
                            off = HALO - (KTAPS - 1 - k) * dil
                            nc.tensor.matmul(
                                cv, dwd[:, t, k, :], xn[:, t, off:off + Sc],
                                start=(k == 0), stop=(k == KTAPS - 1))
                        h = hp.tile([128, Sc], BF16, tag="h")
                        emit_gelu(h, cv, dwb[:, t:t + 1])
                        for m in range(NMC):
                            nc.tensor.matmul(
                                cps[m], cw[:, t, m * 128:(m + 1) * 128], h,
                                start=(t == 0), stop=(t == NT - 1))
                    xn_prev = xn

                    hc = hcp.tile([128, NTE, Sc], BF16, tag="hc")
                    for m in range(NMC):
                        emit_gelu(hc[:, m, :], cps[m], cb[:, m:m + 1])
                    if new_delta is not None:
                        for e in range(NTE):
                            nc.tensor.matmul(
                                new_delta[bk][row:row + 1, :], ecs[:, e:e + 1],
                                hc[:, e, :], start=(e == 0), stop=(e == NTE - 1),
                                tile_position=(0, row))
                    for mo in range(NME):
                        ep = psp.tile([128, Sc], F32, tag="ps")
                        for e in range(NTE):
                            nc.tensor.matmul(
                                ep, ew[:, e, mo * 128:(mo + 1) * 128], hc[:, e, :],
                                start=(e == 0), stop=(e == NTE - 1))
                        nc.vector.scalar_tensor_tensor(
                            xres[mo][:, lo:lo + Sc], ep, eb[:, mo:mo + 1],
                            xres[mo][:, lo:lo + Sc], op0=OP.add, op1=OP.add)
                delta_banks = new_delta

            for t in range(NT):
                nc.sync.dma_start(
                    out=yt_d.ap()[t * 128:(t + 1) * 128, :], in_=xres[t])

    nc.compile()
    return nc


def host_prep(ln_scale, ln_bias, dw_w, dw_b, comp_w, comp_b, exp_w, exp_b):
    """Fold LN affine into conv weights and lay everything out device-friendly."""
    ln_scale = np.asarray(ln_scale, np.float32)
    ln_bias = np.asarray(ln_bias, np.float32)
    dw_w = np.asarray(dw_w, np.float32)
    dw_b = np.asarray(dw_b, np.float32)
    comp_w = np.asarray(comp_w, np.float32)
    comp_b = np.asarray(comp_b, np.float32)
    exp_w = np.asarray(exp_w, np.float32)
    exp_b = np.asarray(exp_b, np.float32)

    dww = dw_w * ln_scale[:, :, None]                       # [L, D, K]
    dwb = dw_b + ln_bias * dw_w.sum(-1)                     # [L, D]
    bf = ml_dtypes.bfloat16
    # diagonal conv weights: dwd[l, p, t, k, p] = dww[l, t*128+p, k]
    dww_ptk = dww.reshape(L, NT, 128, KTAPS).transpose(0, 2, 1, 3)  # [L,128,NT,K]
    dwd = np.zeros((L, 128, NT, KTAPS, 128), np.float32)
    idx = np.arange(128)
    dwd[:, idx, :, :, idx] = dww_ptk.transpose(1, 0, 2, 3)
    ecs = exp_w.sum(1)                                      # [L, DB]
    # ebs[l] is consumed at layer l for the delta produced by layer l-1's
    # expand, so shift the per-layer bias sums by one layer.
    ebs = np.concatenate([[0.0], exp_b.sum(-1)[:-1] / D]).astype(np.float32)
    return {
        "dwd": np.ascontiguousarray(dwd).astype(bf),
        "dwb": np.ascontiguousarray(dwb.reshape(L, NT, 128).transpose(0, 2, 1)),
        "cw": np.ascontiguousarray(
            comp_w.transpose(0, 2, 1).reshape(L, NT, 128, DB)
            .transpose(0, 2, 1, 3)).astype(bf),
        "cb": np.ascontiguousarray(comp_b.reshape(L, NMC, 128).transpose(0, 2, 1)),
        "ew": np.ascontiguousarray(
            exp_w.transpose(0, 2, 1).reshape(L, NTE, 128, D)
            .transpose(0, 2, 1, 3)).astype(bf),
        "eb": np.ascontiguousarray(exp_b.reshape(L, NME, 128).transpose(0, 2, 1)),
        "ecs": np.ascontiguousarray(ecs.reshape(L, NTE, 128).transpose(0, 2, 1))
        .astype(bf),
        "ebs": np.broadcast_to(ebs[:, None, None], (L, 128, 1)).copy(),
    }


_CACHE = {}


def _get_program():
    if "nc" not in _CACHE:
        _CACHE["nc"] = build_program()
    return _CACHE["nc"]


def kernel(**inputs):
    x = np.asarray(inputs["x"], np.float32)                 # [B, S, D]
    w = host_prep(
        inputs["ln_scale"], inputs["ln_bias"], inputs["dw_w"], inputs["dw_b"],
        inputs["comp_w"], inputs["comp_b"], inputs["exp_w"], inputs["exp_b"])
    in_maps = []
    for core in range(B):
        m = dict(w)
        m["xt"] = np.ascontiguousarray(x[core].T)           # [D, S]
        in_maps.append(m)
    res = run_bass_kernel_spmd(_get_program(), in_maps, list(range(B)))
    return np.stack([res.results[i]["yt"].T for i in range(B)], axis=0)
